# revision 1
# baseline (speedup 1.0000x reference)
"""AlignmentTable kernel for 8 Trainium2 NeuronCores.

Reference computation (N1 = N2 = 8192, VOCAB = 1024):
    eq[i,j]   = seq1[i] == seq2[j]
    ch0[i,j]  = eq ? pw_scores[seq1[i], seq2[j]] : 0        (padded to 8193x8193)
    out       = stack([ch0, gap, gap], axis=-1)             (8193, 8193, 3) f32

Where eq holds, pw_scores[seq1[i], seq2[j]] == pw_scores[v, v] — a diagonal
element — so the device only needs dval[i] = diag(pw_scores)[seq1[i]]:
    out[i,j,0] = (seq1[i] == seq2[j]) * dval[i]

Sharding: rows split across 8 cores (1024 rows each = 8 tiles of 128
partitions); seq2 replicated. Each core materializes its (1024, 8193, 3)
slab (~100 MB) — the kernel is a pure HBM-write problem (~805 MB total),
bounded by the SBUF-AXI / HBM write bandwidth.

Per core:
  * seq2 (+ one -1 pad column) is broadcast across the 128 partitions via
    a TensorEngine ones-column matmul (PSUM) + ScalarE copies to SBUF —
    keeping the replication off the DMA engines, which are 100% busy
    storing output.
  * three interleaved row buffers (128, 3*4097) are filled once with
    gap_score; only the stride-3 channel-0 slots are rewritten afterwards.
  * per (row-tile, column-chunk): one VectorE tensor_scalar
        (seq2_chunk == tok_row) * dval_row
    into the stride-3 ch0 slots, then one ~6.3 MB contiguous HWDGE DMA
    of the interleaved buffer to HBM.
The trailing output row 8192 (constant) is written on the host.
"""

import numpy as np

N1 = 8192
N2 = 8192
NCORES = 8
P = 128
ROWS_PER_CORE = N1 // NCORES          # 1024
RTILES = ROWS_PER_CORE // P           # 8
NJ = N2 + 1                           # 8193 output columns
# Row-tile 0 uses small leading chunks so the first store starts ASAP;
# later row-tiles use two big chunks (~6.3 MB DMAs).
CHUNKS0 = ((0, 1024), (1024, 1025), (2049, 2048), (4097, 4096))
CHUNKS = ((0, 4097), (4097, 4096))
BUFW = 3 * 4097                       # 12291 interleaved f32 per partition
NBUF = 3
MMW = 512                             # matmul free-dim width (one PSUM bank)
_cache = {}


def _build_nc():
    import concourse.bacc as bacc
    import concourse.mybir as mybir
    from concourse.tile import TileContext

    f32 = mybir.dt.float32
    f16 = mybir.dt.float16
    nc = bacc.Bacc(None, target_bir_lowering=False)

    # meta columns: [0:8] tok per row-tile, [8:16] dval per row-tile, [16] gap
    meta = nc.dram_tensor("meta", [P, 2 * RTILES + 1], f32, kind="ExternalInput")
    # seq2 tokens in fp16 (0..1023 and the -1 pad are exact) so the
    # broadcast matmuls run at fp16 PE speed.
    s2 = nc.dram_tensor("s2", [NJ], f16, kind="ExternalInput")
    out = nc.dram_tensor("out", [ROWS_PER_CORE, 3 * NJ], f32, kind="ExternalOutput")

    with TileContext(nc) as tc:
        with (
            tc.tile_pool(name="sbuf", bufs=1) as pool,
            tc.tile_pool(name="psum", bufs=2, space="PSUM") as psum,
        ):
            META = pool.tile([P, 2 * RTILES + 1], f32, tag="meta")
            ONES = pool.tile([1, P], f16, tag="ones")
            S2ROW = pool.tile([1, NJ], f16, tag="s2row")
            S2B = pool.tile([P, NJ], f32, tag="s2b")
            BUFS = [
                pool.tile([P, BUFW], f32, tag=f"buf{i}", name=f"buf{i}")
                for i in range(NBUF)
            ]
            GAP = META[:, 2 * RTILES : 2 * RTILES + 1]

            # Parallel input loads: seq2 row via HWDGE, meta via SWDGE.
            nc.sync.dma_start(out=S2ROW[:], in_=s2[None, :])
            nc.gpsimd.dma_start(out=META[:], in_=meta[:])
            nc.vector.memset(ONES[:], 1.0)

            # Broadcast seq2 across partitions: S2B[p, j] = s2[j] via
            # ones(128) outer-product matmuls, copied PSUM -> SBUF on ACT.
            for k in range((NJ + MMW - 1) // MMW):
                lo = k * MMW
                w = min(MMW, NJ - lo)
                ps = psum.tile([P, MMW], f32, tag="ps", name="ps")
                nc.tensor.matmul(
                    ps[:, :w], ONES[:], S2ROW[:, lo : lo + w], start=True, stop=True
                )
                nc.scalar.copy(out=S2B[:, lo : lo + w], in_=ps[:, :w])

            def fill(b, lo, hi, after):
                # Gap fill of [lo, hi); only the stride-3 ch1/ch2 slots
                # survive, ch0 slots are rewritten before every DMA out.
                # `after` (the previous chunk's buffer, or None) is read as
                # 0*x + gap purely to order this fill behind that chunk's
                # compute — otherwise the scheduler front-loads every fill
                # before the first store.
                if after is None:
                    nc.vector.tensor_scalar(
                        out=b[:, lo:hi],
                        in0=GAP.to_broadcast((P, hi - lo)),
                        scalar1=1.0,
                        scalar2=None,
                        op0=mybir.AluOpType.mult,
                    )
                else:
                    nc.vector.tensor_scalar(
                        out=b[:, lo:hi],
                        in0=after[:, 0:1].to_broadcast((P, hi - lo)),
                        scalar1=0.0,
                        scalar2=GAP,
                        op0=mybir.AluOpType.mult,
                        op1=mybir.AluOpType.add,
                    )

            filled = [0] * NBUF
            bi = 0
            for rt in range(RTILES):
                for cs, w in CHUNKS0 if rt == 0 else CHUNKS:
                    k = bi % NBUF
                    b = BUFS[k]
                    prev = BUFS[(bi - 1) % NBUF] if bi > 0 else None
                    bi += 1
                    need = 3 * w
                    if filled[k] < need:
                        # first fill: exactly what's needed (fast start);
                        # second: finish the buffer for all later reuses.
                        hi = need if filled[k] == 0 else BUFW
                        fill(b, filled[k], hi, prev)
                        filled[k] = hi
                    nc.vector.tensor_scalar(
                        out=b[:, 0 : 3 * w : 3],
                        in0=S2B[:, cs : cs + w],
                        scalar1=META[:, rt : rt + 1],
                        scalar2=META[:, RTILES + rt : RTILES + rt + 1],
                        op0=mybir.AluOpType.is_equal,
                        op1=mybir.AluOpType.mult,
                    )
                    nc.sync.dma_start(
                        out=out[rt * P : (rt + 1) * P, 3 * cs : 3 * (cs + w)],
                        in_=b[:, : 3 * w],
                    )
    nc.compile()
    return nc


def _get_nc():
    if "nc" not in _cache:
        _cache["nc"] = _build_nc()
    return _cache["nc"]


def _prep_in_maps(encoded_seq1, encoded_seq2, pw_scores, gap_score):
    seq1 = np.asarray(encoded_seq1).astype(np.int64)
    seq2 = np.asarray(encoded_seq2).astype(np.int64)
    pw = np.asarray(pw_scores).astype(np.float32)
    gapf = np.float32(np.asarray(gap_score))

    dvals = pw.diagonal().astype(np.float32)[seq1]      # (8192,)
    seq1f = seq1.astype(np.float32)
    s2pad = np.empty(NJ, np.float16)
    s2pad[:N2] = seq2.astype(np.float16)                # 0..1023: exact in fp16
    s2pad[N2] = -1.0                                    # never matches a token

    in_maps = []
    for r in range(NCORES):
        lo, hi = r * ROWS_PER_CORE, (r + 1) * ROWS_PER_CORE
        meta = np.empty((P, 2 * RTILES + 1), np.float32)
        meta[:, :RTILES] = seq1f[lo:hi].reshape(RTILES, P).T
        meta[:, RTILES : 2 * RTILES] = dvals[lo:hi].reshape(RTILES, P).T
        meta[:, 2 * RTILES] = gapf
        in_maps.append({"s2": s2pad, "meta": meta})
    return in_maps, gapf


def _assemble(results, gapf):
    out = np.empty((N1 + 1, NJ, 3), np.float32)
    for r in range(NCORES):
        out[r * ROWS_PER_CORE : (r + 1) * ROWS_PER_CORE] = results[r]["out"].reshape(
            ROWS_PER_CORE, NJ, 3
        )
    out[N1, :, 0] = 0.0
    out[N1, :, 1] = gapf
    out[N1, :, 2] = gapf
    return out


def run(encoded_seq1, encoded_seq2, pw_scores, gap_score, **spmd_kwargs):
    """Full pipeline; extra kwargs (trace=..., tmpdir=...) are forwarded to
    run_bass_kernel_spmd. Returns (output, BassKernelResults)."""
    from concourse.bass_utils import run_bass_kernel_spmd

    in_maps, gapf = _prep_in_maps(encoded_seq1, encoded_seq2, pw_scores, gap_score)
    res = run_bass_kernel_spmd(
        _get_nc(), in_maps, core_ids=list(range(NCORES)), **spmd_kwargs
    )
    return _assemble(res.results, gapf), res


def kernel(encoded_seq1, encoded_seq2, pw_scores, gap_score):
    out, _ = run(encoded_seq1, encoded_seq2, pw_scores, gap_score)
    return out

